# revision 1
# baseline (speedup 1.0000x reference)
"""Trainium2 Bass kernel for nn_AffineLayer (topk_masking):
out[b, f] = max_p(x[b] . ww[f, p]) * scale[f] + bias[f]

Shapes (hardcoded per problem spec):
  x     (2048, 1, 8, 8)  -> xf (2048, 64)
  ww    (1024, 64, 1, 8, 8) -> wwf (1024, 64, 64)   (f, p, i)
  scale (1, 1024), bias (1, 1024)
  out   (2048, 1024)

Sharding: f tensor-parallel over 8 cores (f_shard = 128 per core), x replicated.

Per-core device layout (f on partitions):
  lhsT (stationary) = wT[:, p, :] : (i=64, f=128)  per p-plane
  rhs  (moving)     = xT[:, bchunk]: (i=64, b=512)
  psum out          = (f=128, b=512) per p-plane, 1 PSUM bank

The 64-way max over p is the bottleneck: every score must leave PSUM through
one of the only two PSUM-capable engines (DVE and ACT, both 1 elem/cycle/lane
for fp32). p-plane groups are split between:
  - DVE: running tensor_tensor(max) straight from PSUM into a 4-slot fp32 acc
  - ACT: activation(Copy) PSUM -> SBUF staging (cast to fp16: same 16-bit
    2x fold speed as bf16, 8x the mantissa precision), folded into 16-bit
    accumulators by DVE tensor_tensor at 2x packed rate.
Final per-chunk: fold acc slots, combine paths, apply scale/bias via one
tensor_scalar with per-partition (f) scalars, DMA out as (128f, 2048b).
Host reassembles and transposes to (2048, 1024).
"""

import os
import sys

if "/opt/trn_rl_repo" not in sys.path:
    sys.path.insert(0, "/opt/trn_rl_repo")

import numpy as np

import concourse.bass as bass
import concourse.mybir as mybir
from concourse.tile import TileContext
from concourse.bass_utils import run_bass_kernel_spmd

# Problem dims (hardcoded)
B, FDIM, P, IDIM = 2048, 1024, 64, 64
N_CORES = 8
F_SH = FDIM // N_CORES  # 128
BCH = 512  # b-chunk size (PSUM bank = 512 fp32)
NJ = B // BCH  # 4
GQ = int(os.environ.get("KGQ", "2"))  # p-planes per group (= PSUM banks)
NG = P // GQ  # groups
PSUM_BUFS = 8 // GQ

# ---- Tunables ----------------------------------------------------------
# Per-group drain assignment, length NG. "D" = DVE direct TT-max from PSUM;
# "V" = ACT copy -> staged, folded by DVE; "G" = ACT copy -> staged, folded
# by GPSIMD.
ASSIGN = os.environ.get("KASSIGN", "VVDVVVDVVVDVVVDVVVDVVVDVVVDVVVDV")
STAGE_BF16 = os.environ.get("KSTAGE_BF16", "1") == "1"
# Staging dtype: fp16 matches bf16's 2x DVE fold speed (both 16-bit) but has
# 10 mantissa bits vs 7 — scores (|s| < ~70) sit far inside fp16 range.
STAGE_DT_NAME = os.environ.get("KSTAGE_DT", "float16" if STAGE_BF16 else "float32")
# Matmul input dtype: float32r streams 1 row/cycle on the PE (vs 4 for fp32,
# which decomposes into 2 half-speed passes); same 4-byte layout as fp32.
MM_DT_NAME = os.environ.get("KMM_DT", "float32r")
# Unified accumulator: direct-drained groups also max into the bf16 staged
# acc (drops the separate fp32 acc + its tail folds; whole output ~bf16).
UNIFIED = os.environ.get("KUNIFIED", "0") == "1"
NWCH = int(os.environ.get("KNWCH", "32"))
REPS = int(os.environ.get("KREPS", "0"))  # >0: wrap body in a For_i repeat loop (bench only)
STAGE_BUFS = int(os.environ.get("KSTAGE_BUFS", "6"))
XT_CHUNKED = os.environ.get("KXT_CHUNKED", "1") == "1"
DQUAD = os.environ.get("KDQUAD", "0") == "1"  # D-groups drain as 4-bank quads
JINT = os.environ.get("KJINT", "0") == "1"  # interleave all b-chunks per p-position
# ------------------------------------------------------------------------

F32 = mybir.dt.float32
BF16 = mybir.dt.bfloat16
STAGE_DT = getattr(mybir.dt, STAGE_DT_NAME)
MM_DT = getattr(mybir.dt, MM_DT_NAME)
MX = mybir.AluOpType.max


def split_multiwaits(nc):
    """This walrus build allows at most ONE sem wait per instruction.
    Tile's wait assignment can emit several; hoist extras onto inserted
    sequencer nops immediately before the over-subscribed instruction
    (same engine, program order preserved => identical semantics)."""
    wid = 0
    for f in nc.m.functions:
        for bb in f.blocks:
            il = bb.instructions
            i = 0
            while i < len(il):
                ins = il[i]
                si = getattr(ins, "sync_info", None)
                if si is not None and si.on_wait and len(si.on_wait) > 1:
                    waits = list(si.on_wait)
                    si.on_wait = waits[-1:]
                    carriers = []
                    for w in waits[:-1]:
                        wid += 1
                        carriers.append(
                            mybir.InstNoOp(
                                name=f"WSPLIT-{wid}",
                                engine=ins.engine,
                                sync_info=mybir.SyncInfo(on_wait=[w], on_update=[]),
                            )
                        )
                    il[i:i] = carriers
                    i += len(carriers)
                i += 1


def build_nc_jint(assign=None, fixup=True, affine=True):
    """b-chunk-interleaved variant: iterate p-positions outer, all NJ b-chunks
    inner. Staged tiles hold one position x all chunks (NJ*GQ planes), folded
    by one DVE TT; accumulators span all chunks so the tails and the output
    DMA are whole-row ops."""
    assign = (assign or ASSIGN).split(";")[0]
    assert len(assign) in (16, NG) and set(assign) <= set("DV")
    if len(assign) != NG:
        assign = "".join(c * (NG // 16) for c in assign)
    last_d = assign.rfind("D")
    last_v = assign.rfind("V")

    nc = bass.Bass()
    xt_d = nc.dram_tensor("xt", [IDIM, B], MM_DT, kind="ExternalInput")
    wt_d = nc.dram_tensor("wt", [IDIM, P, F_SH], MM_DT, kind="ExternalInput")
    sc_d = nc.dram_tensor("scale", [F_SH, 1], F32, kind="ExternalInput")
    bi_d = nc.dram_tensor("bias", [F_SH, 1], F32, kind="ExternalInput")
    y_d = nc.dram_tensor("y", [F_SH, B], F32, kind="ExternalOutput")

    PW = P // NWCH

    with TileContext(nc) as tc:
        with (
            tc.tile_pool(name="const", bufs=1) as const,
            tc.tile_pool(name="psum", bufs=PSUM_BUFS, space="PSUM") as psum,
            tc.tile_pool(name="accs", bufs=2) as accs,
            tc.tile_pool(name="stage", bufs=STAGE_BUFS) as stage,
            tc.tile_pool(name="outs", bufs=2) as outs,
        ):
            xt = const.tile([IDIM, B], MM_DT)
            nc.sync.dma_start(out=xt[:, 0:BCH], in_=xt_d[:, 0:BCH])
            wchunks = [
                const.tile([IDIM, PW, F_SH], MM_DT, name=f"wt{c}") for c in range(NWCH)
            ]
            nc.sync.dma_start(out=wchunks[0][:], in_=wt_d[:, 0:PW, :])
            for c in range(1, NJ):
                nc.sync.dma_start(
                    out=xt[:, c * BCH : (c + 1) * BCH],
                    in_=xt_d[:, c * BCH : (c + 1) * BCH],
                )
            for c in range(1, NWCH):
                nc.sync.dma_start(
                    out=wchunks[c][:], in_=wt_d[:, c * PW : (c + 1) * PW, :]
                )
            sc = const.tile([F_SH, 1], F32)
            nc.sync.dma_start(out=sc[:], in_=sc_d[:])
            bi = const.tile([F_SH, 1], F32)
            nc.sync.dma_start(out=bi[:], in_=bi_d[:])
            warm = const.tile([F_SH, 2], F32)
            nc.vector.memset(warm[:], 0.0)
            nc.scalar.activation(
                out=warm[:, 1:2], in_=warm[:, 0:1],
                func=mybir.ActivationFunctionType.Copy,
            )

            import contextlib

            loop_cm = (
                tc.For_i(0, REPS, 1, hint_engines=(mybir.EngineType.PE,))
                if REPS > 0
                else contextlib.nullcontext()
            )
            with loop_cm:
                acc_d = accs.tile([F_SH, NJ, GQ, BCH], F32, tag="acc_d")
                acc_v = accs.tile([F_SH, NJ, GQ, BCH], STAGE_DT, tag="acc_v")
                n_d = n_v = 0

                def fold_gq(acc):
                    w = GQ
                    while w > 1:
                        h = w // 2
                        nc.vector.tensor_max(
                            acc[:, :, 0:h, :], acc[:, :, 0:h, :], acc[:, :, h:w, :]
                        )
                        w = h

                for g in range(NG):
                    if assign[g] == "D":
                        for j in range(NJ):
                            pt = psum.tile([F_SH, GQ, BCH], F32, tag="ps")
                            for q in range(GQ):
                                p = GQ * g + q
                                nc.tensor.matmul(
                                    pt[:, q, :],
                                    wchunks[p // PW][:, p % PW, :],
                                    xt[:, j * BCH : (j + 1) * BCH],
                                    start=True,
                                    stop=True,
                                )
                            dst = acc_d[:, j]
                            if n_d == 0:
                                nc.vector.tensor_copy(out=dst, in_=pt[:])
                            else:
                                nc.vector.tensor_max(dst, pt[:], dst)
                        n_d += 1
                        if g == last_d and last_d > last_v:
                            fold_gq(acc_d)
                    else:
                        st = stage.tile([F_SH, NJ, GQ, BCH], STAGE_DT, tag="st")
                        for j in range(NJ):
                            pt = psum.tile([F_SH, GQ, BCH], F32, tag="ps")
                            for q in range(GQ):
                                p = GQ * g + q
                                nc.tensor.matmul(
                                    pt[:, q, :],
                                    wchunks[p // PW][:, p % PW, :],
                                    xt[:, j * BCH : (j + 1) * BCH],
                                    start=True,
                                    stop=True,
                                )
                            nc.scalar.activation(
                                out=st[:, j],
                                in_=pt[:],
                                func=mybir.ActivationFunctionType.Copy,
                            )
                        if n_v == 0:
                            nc.vector.tensor_copy(out=acc_v[:], in_=st[:])
                        else:
                            nc.vector.tensor_max(acc_v[:], st[:], acc_v[:])
                        n_v += 1
                        if g == last_v and last_v > last_d:
                            fold_gq(acc_v)

                # ---- tails: whole-row ops across all chunks ------------
                if n_v and last_v < last_d:
                    fold_gq(acc_v)
                if n_d and last_d < last_v:
                    fold_gq(acc_d)
                staged = acc_v[:, :, 0, :] if n_v else None  # (F_SH, NJ, BCH)
                direct = acc_d[:, :, 0, :] if n_d else None
                outt = outs.tile([F_SH, NJ, BCH], F32, tag="outt")
                if direct is not None and staged is not None:
                    nc.vector.tensor_max(outt[:], direct, staged)
                    src = outt[:]
                elif direct is not None:
                    src = direct
                else:
                    src = staged
                if affine:
                    nc.vector.tensor_scalar(
                        out=outt[:],
                        in0=src,
                        scalar1=sc[:],
                        scalar2=bi[:],
                        op0=mybir.AluOpType.mult,
                        op1=mybir.AluOpType.add,
                    )
                    src = outt[:]
                elif src is not outt[:] and src.dtype != F32:
                    nc.vector.tensor_copy(out=outt[:], in_=src)
                    src = outt[:]
                nc.sync.dma_start(out=y_d[:], in_=src)

    if fixup:
        split_multiwaits(nc)
    return nc



def build_nc(assign=None, fixup=True, affine=True):
    if JINT:
        return build_nc_jint(assign=assign, fixup=fixup, affine=affine)
    assign = assign or ASSIGN
    pats = assign.split(";")
    if len(pats) == 1:
        pats = pats * NJ
    assert len(pats) == NJ
    expanded = []
    for p_ in pats:
        assert len(p_) in (16, NG) and set(p_) <= set("DV")
        if len(p_) != NG:
            p_ = "".join(c * (NG // 16) for c in p_)
        expanded.append(p_)
    pats = expanded

    nc = bass.Bass()
    xt_d = nc.dram_tensor("xt", [IDIM, B], MM_DT, kind="ExternalInput")
    wt_d = nc.dram_tensor("wt", [IDIM, P, F_SH], MM_DT, kind="ExternalInput")
    sc_d = nc.dram_tensor("scale", [F_SH, 1], F32, kind="ExternalInput")
    bi_d = nc.dram_tensor("bias", [F_SH, 1], F32, kind="ExternalInput")
    y_d = nc.dram_tensor("y", [F_SH, B], F32, kind="ExternalOutput")

    PW = P // NWCH  # p-planes per weight chunk
    VS = 2 * GQ  # staged-pair slot count (2 groups per staged tile)

    with TileContext(nc) as tc:
        with (
            tc.tile_pool(name="const", bufs=1) as const,
            tc.tile_pool(name="psum", bufs=PSUM_BUFS, space="PSUM") as psum,
            tc.tile_pool(
                name="accs", bufs=int(os.environ.get("KACC_BUFS", "2"))
            ) as accs,
            tc.tile_pool(name="stage", bufs=STAGE_BUFS) as stage,
            tc.tile_pool(
                name="outs", bufs=int(os.environ.get("KOUT_BUFS", "2"))
            ) as outs,
        ):
            # input loads: first-needed chunks first so group 0 starts ASAP
            xt = const.tile([IDIM, B], MM_DT)
            wchunks = [
                const.tile([IDIM, PW, F_SH], MM_DT, name=f"wt{c}") for c in range(NWCH)
            ]
            nc.sync.dma_start(out=xt[:, 0:BCH], in_=xt_d[:, 0:BCH])
            nc.sync.dma_start(out=wchunks[0][:], in_=wt_d[:, 0:PW, :])
            nc.sync.dma_start(out=wchunks[1][:], in_=wt_d[:, PW : 2 * PW, :])
            for c in range(2, NWCH):
                nc.sync.dma_start(
                    out=wchunks[c][:], in_=wt_d[:, c * PW : (c + 1) * PW, :]
                )
            for c in range(1, NJ):
                nc.sync.dma_start(
                    out=xt[:, c * BCH : (c + 1) * BCH],
                    in_=xt_d[:, c * BCH : (c + 1) * BCH],
                )
            sc = const.tile([F_SH, 1], F32)
            nc.sync.dma_start(out=sc[:], in_=sc_d[:])
            bi = const.tile([F_SH, 1], F32)
            nc.sync.dma_start(out=bi[:], in_=bi_d[:])
            warm = const.tile([F_SH, 2], F32)
            nc.vector.memset(warm[:], 0.0)
            nc.scalar.activation(
                out=warm[:, 1:2], in_=warm[:, 0:1],
                func=mybir.ActivationFunctionType.Copy,
            )

            import contextlib

            loop_cm = (
                tc.For_i(0, REPS, 1, hint_engines=(mybir.EngineType.PE,))
                if REPS > 0
                else contextlib.nullcontext()
            )
            with loop_cm:
              for j in range(NJ):
                assign_j = pats[j]
                last_d = assign_j.rfind("D")
                rhs = xt[:, j * BCH : (j + 1) * BCH]
                DS = 4 if DQUAD else GQ
                acc_d = accs.tile([F_SH, DS, BCH], F32, tag="acc_d")
                acc_v = accs.tile([F_SH, VS, BCH], STAGE_DT, tag="acc_v")
                n_d = n_v = 0
                half = 0  # staged-pair fill state
                st = None

                def flush_pair(full):
                    nonlocal n_v, st
                    if full:
                        src = st[:].rearrange("p a g b -> p (a g) b")
                        dst = acc_v[:]
                    else:
                        src = st[:, 0]
                        dst = acc_v[:, 0:GQ, :]
                    if n_v == 0:
                        nc.vector.tensor_copy(out=dst, in_=src)
                    else:
                        nc.vector.tensor_max(dst, src, dst)
                    n_v += 1
                    st = None

                # build token schedule: D-pairs become 4-bank quads in DQUAD mode
                tokens = []
                g = 0
                while g < NG:
                    if (
                        DQUAD
                        and assign_j[g] == "D"
                    ):
                        assert g + 1 < NG and assign_j[g + 1] == "D", (
                            "KDQUAD=1 requires D groups in adjacent pairs"
                        )
                        tokens.append(("D", g, 2 * GQ))
                        g += 2
                    else:
                        tokens.append((assign_j[g], g, GQ))
                        g += 1
                n_dtok = sum(1 for t in tokens if t[0] == "D")
                dtok_i = 0
                for kind, g0, nplanes in tokens:
                    if kind == "D" and DQUAD:
                        pt = psum.tile([F_SH, 4, BCH], F32, tag="psd", bufs=1, name="ptd")
                    else:
                        pt = psum.tile(
                            [F_SH, GQ, BCH],
                            F32,
                            tag="ps",
                            bufs=2 if DQUAD else PSUM_BUFS,
                            name="ptv",
                        )
                    for q in range(nplanes):
                        p = GQ * g0 + q
                        nc.tensor.matmul(
                            pt[:, q, :],
                            wchunks[p // PW][:, p % PW, :],
                            rhs,
                            start=True,
                            stop=True,
                        )
                    if kind == "D":
                        dst = acc_d[:] if nplanes == DS else acc_d[:, 0:nplanes, :]
                        if n_d == 0:
                            assert nplanes == DS, "first D token must fill acc_d"
                            nc.vector.tensor_copy(out=dst, in_=pt[:])
                        else:
                            nc.vector.tensor_max(dst, pt[:], dst)
                        n_d += 1
                        dtok_i += 1
                        if dtok_i == n_dtok:
                            w = DS
                            while w > 1:
                                h = w // 2
                                nc.vector.tensor_max(
                                    acc_d[:, 0:h, :],
                                    acc_d[:, 0:h, :],
                                    acc_d[:, h:w, :],
                                )
                                w = h
                    else:
                        if st is None:
                            st = stage.tile([F_SH, 2, GQ, BCH], STAGE_DT, tag="st")
                        nc.scalar.activation(
                            out=st[:, half],
                            in_=pt[:],
                            func=mybir.ActivationFunctionType.Copy,
                        )
                        half ^= 1
                        if half == 0:
                            flush_pair(full=True)
                if half == 1:
                    flush_pair(full=False)

                # ---- tails ------------------------------------------------
                staged = None
                if n_v:
                    w = VS
                    while w > 1:
                        h = w // 2
                        nc.vector.tensor_max(
                            acc_v[:, 0:h, :], acc_v[:, 0:h, :], acc_v[:, h:w, :]
                        )
                        w = h
                    staged = acc_v[:, 0, :]
                direct = acc_d[:, 0, :] if n_d else None

                outt = outs.tile([F_SH, BCH], F32, tag="outt")
                if direct is not None and staged is not None:
                    nc.vector.tensor_max(outt[:], direct, staged)  # mixed dtype OK
                    src = outt[:]
                elif direct is not None:
                    src = direct
                else:
                    src = staged
                if affine:
                    nc.vector.tensor_scalar(
                        out=outt[:],
                        in0=src,
                        scalar1=sc[:],
                        scalar2=bi[:],
                        op0=mybir.AluOpType.mult,
                        op1=mybir.AluOpType.add,
                    )
                    src = outt[:]
                elif src is not outt[:] and src.dtype != F32:
                    nc.vector.tensor_copy(out=outt[:], in_=src)
                    src = outt[:]
                nc.sync.dma_start(out=y_d[:, j * BCH : (j + 1) * BCH], in_=src)

    if fixup:
        split_multiwaits(nc)
    return nc


_CACHED_NC = None


def _get_nc():
    global _CACHED_NC
    if _CACHED_NC is None:
        _CACHED_NC = build_nc()
    return _CACHED_NC


def make_in_maps(x, ww, scale, bias):
    x = np.asarray(x)
    ww = np.asarray(ww)
    scale = np.asarray(scale)
    bias = np.asarray(bias)

    xf = np.ascontiguousarray(x.reshape(B, IDIM).T).astype(np.float32)  # (64, 2048)
    wwf = ww.reshape(FDIM, P, IDIM)
    sc = scale.reshape(FDIM).astype(np.float32)
    bi = bias.reshape(FDIM).astype(np.float32)

    in_maps = []
    for k in range(N_CORES):
        wk = wwf[k * F_SH : (k + 1) * F_SH]  # (128, 64, 64) = (f, p, i)
        wt = np.ascontiguousarray(wk.transpose(2, 1, 0)).astype(np.float32)  # (i,p,f)
        in_maps.append(
            {
                "xt": xf,
                "wt": wt,
                "scale": np.ascontiguousarray(
                    sc[k * F_SH : (k + 1) * F_SH].reshape(F_SH, 1)
                ),
                "bias": np.ascontiguousarray(
                    bi[k * F_SH : (k + 1) * F_SH].reshape(F_SH, 1)
                ),
            }
        )
    return in_maps


def kernel(x, ww, scale, bias):
    in_maps = make_in_maps(x, ww, scale, bias)
    trivial_affine = bool(
        np.all(np.asarray(scale) == 1.0) and np.all(np.asarray(bias) == 0.0)
    )
    nc = build_nc(affine=not trivial_affine)
    res = run_bass_kernel_spmd(nc, in_maps, list(range(N_CORES)))
    out = np.empty((FDIM, B), dtype=np.float32)
    for k in range(N_CORES):
        out[k * F_SH : (k + 1) * F_SH] = res.results[k]["y"]
    return np.ascontiguousarray(out.T)



# revision 20
# speedup vs baseline: 1.1057x; 1.1057x over previous
"""Trainium2 Bass kernel for nn_AffineLayer (topk_masking):
out[b, f] = max_p(x[b] . ww[f, p]) * scale[f] + bias[f]

Shapes (hardcoded per problem spec):
  x     (2048, 1, 8, 8)  -> xf (2048, 64)
  ww    (1024, 64, 1, 8, 8) -> wwf (1024, 64, 64)   (f, p, i)
  scale (1, 1024), bias (1, 1024)
  out   (2048, 1024)

Sharding: f tensor-parallel over 8 cores (f_shard = 128 per core), x replicated.

Per-core device layout (f on partitions):
  lhsT (stationary) = wT[:, p, :] : (i=64, f=128)  per p-plane
  rhs  (moving)     = xT[:, bchunk]: (i=64, b=512)
  psum out          = (f=128, b=512) per p-plane, GQ planes per PSUM tile

The 64-way max over p is the bottleneck: every score must leave PSUM through
one of the only two PSUM-capable engines (DVE @0.96GHz and ACT @1.2GHz, both
1 elem/cycle/lane for fp32 reads). The drain work is split:
  - "D" groups: DVE tensor_tensor(max) straight from PSUM into an fp16 acc
    (fold fused into the drain).
  - "V" groups: ACT activation(Copy) PSUM -> SBUF fp16 staging. Staged pairs
    are then max-folded into per-engine fp16 accumulators by POOL
    (gpsimd tensor_max) and by DMA (SDMA CCE accum_op=max) -- engines that
    are otherwise idle -- so DVE's cycles stay dedicated to PSUM drains.
Tail folds / accumulator merges also ride the DMA CCE path; the final affine
runs on DVE as a 4x-mode fp16 tensor_scalar. Output is stored fp16 and
upconverted on host (fp16 rounding ~2e-4 rel err, far inside the 2e-2 gate).
"""

import os
import sys

if "/opt/trn_rl_repo" not in sys.path:
    sys.path.insert(0, "/opt/trn_rl_repo")

import numpy as np

import concourse.bass as bass
import concourse.mybir as mybir
from concourse.tile import TileContext
from concourse.bass_utils import run_bass_kernel_spmd

# Problem dims (hardcoded)
B, FDIM, P, IDIM = 2048, 1024, 64, 64
N_CORES = 8
F_SH = FDIM // N_CORES  # 128
BCH = 512  # b-chunk size (PSUM bank = 512 fp32)
NJ = B // BCH  # 4
GQ = int(os.environ.get("KGQ", "2"))  # p-planes per group (= PSUM banks)
NG = P // GQ  # groups per chunk
PSUM_BUFS = 8 // GQ

# ---- Tunables ----------------------------------------------------------
# Per-group drain assignment, length NG (or NG-divisible repeat).
# "D" = DVE direct TT-max from PSUM; "V" = ACT copy -> fp16 staging.
ASSIGN = os.environ.get("KASSIGN", "D" * 14 + "V" * 18)
# Engine per staged-group fold, consumed in V-group order: P=POOL(gpsimd),
# M=DMA(CCE accum max, SWDGE), V=DVE.
# NOTE: "M" (SDMA CCE accum max) is rejected by the compiler verifier
# ("DMACopy does not support max with Copy mode") -- only P/V are usable.
FOLD_PAT = os.environ.get("KFOLD", "PPPV")
ODD_ENG = os.environ.get("KODD", "M")  # engine for a trailing half-pair fold
TAIL_ENG = os.environ.get("KTAIL", "P")  # engine for tail folds & merges
JINT = os.environ.get("KJINT", "1") == "1"  # whole-row accumulators
# Staging dtype: fp16 matches bf16's 2x DVE fold speed (both 16-bit) but has
# 10 mantissa bits vs 7 -- scores (|s| < ~70) sit far inside fp16 range.
STAGE_DT_NAME = os.environ.get("KSTAGE_DT", "float16")
# Matmul input dtype: float32r streams 1 row/cycle on the PE for N>=256.
MM_DT_NAME = os.environ.get("KMM_DT", "float32r")
OUT16 = os.environ.get("KOUT16", "1") == "1"  # store y fp16, upconvert on host
NWCH = int(os.environ.get("KNWCH", "32"))
REPS = int(os.environ.get("KREPS", "0"))  # >0: wrap body in For_i (bench only)
STAGE_BUFS = int(os.environ.get("KSTAGE_BUFS", "4" if os.environ.get("KJINT", "1") == "1" else "5"))
ACC_BUFS = int(os.environ.get("KACC_BUFS", "2"))
# ------------------------------------------------------------------------

F32 = mybir.dt.float32
STAGE_DT = getattr(mybir.dt, STAGE_DT_NAME)
MM_DT = getattr(mybir.dt, MM_DT_NAME)
OUT_DT = STAGE_DT if OUT16 else F32
MX = mybir.AluOpType.max


def split_multiwaits(nc):
    """This walrus build allows at most ONE sem wait per instruction.
    Tile's wait assignment can emit several; hoist extras onto inserted
    sequencer nops immediately before the over-subscribed instruction
    (same engine, program order preserved => identical semantics)."""
    wid = 0
    for f in nc.m.functions:
        for bb in f.blocks:
            il = bb.instructions
            i = 0
            while i < len(il):
                ins = il[i]
                si = getattr(ins, "sync_info", None)
                if si is not None and si.on_wait and len(si.on_wait) > 1:
                    waits = list(si.on_wait)
                    si.on_wait = waits[-1:]
                    carriers = []
                    for w in waits[:-1]:
                        wid += 1
                        carriers.append(
                            mybir.InstNoOp(
                                name=f"WSPLIT-{wid}",
                                engine=ins.engine,
                                sync_info=mybir.SyncInfo(on_wait=[w], on_update=[]),
                            )
                        )
                    il[i:i] = carriers
                    i += len(carriers)
                i += 1


RELU = os.environ.get("KRELU", "1") == "1"
# number of R-units (each = 2 relu-pairs = 4 planes -> 2 maxed planes);
# remaining 64 - 4*UR planes form N-units of 2 planes each.
UR = int(os.environ.get("KUR", "14"))
# per-unit final-drain engine: D = DVE direct TT, V = ACT copy + DVE fold.
# length UR + (64 - 4*UR)//2.
DRAIN = os.environ.get("KDRAIN", "")


def build_nc_relu(fixup=True, affine=True):
    """Relu-pair compression: max(a, b) = b + relu(a - b).

    The 64-way max over p is PSUM-drain-bound: every score crosses
    PSUM->SBUF through ACT (1.2 GHz) or DVE (0.96 GHz) exactly once, and
    every ACT-staged element needs a DVE max-fold (0.5 cyc/elem) -- DVE is
    the ONLY tensor-tensor-max engine (POOL TT doesn't exist in V3 codegen,
    DMA CCE max is rejected). That caps the simple scheme at ~95us.

    This variant converts fold work into PE work (PE is ~50% idle):
    weights are host-differenced (wd = w_even - w_odd), PE emits d-planes,
    ACT drains relu(d) -> fp16 (a crossing it had to do anyway), and PE
    re-injects relu(d) into the base plane's PSUM accumulation group via an
    identity-stationary matmul. Each relu-pair thus max-reduces 2 planes on
    the PE, eliminating the downstream fold for them entirely.

    Injects are software-pipelined one (unit, j) block behind their relu so
    the in-order PE never waits on ACT latency.
    """
    UN = (P - 4 * UR) // 2
    n_units = UR + UN
    drain = DRAIN
    if not drain:
        # spread V-drains evenly, ~1 V per 3.6 units
        nv = max(1, round(n_units * 0.27))
        step = n_units / nv
        vpos = {int(step * (i + 0.5)) for i in range(nv)}
        drain = "".join("V" if u in vpos else "D" for u in range(n_units))
    assert len(drain) == n_units and set(drain) <= set("DV")

    nc = bass.Bass()
    xt_d = nc.dram_tensor("xt", [IDIM, B], MM_DT, kind="ExternalInput")
    wt_d = nc.dram_tensor("wt", [IDIM, P, F_SH], MM_DT, kind="ExternalInput")
    id_d = nc.dram_tensor("ident", [F_SH, F_SH], STAGE_DT, kind="ExternalInput")
    sc_d = nc.dram_tensor("scale", [F_SH, 1], F32, kind="ExternalInput")
    bi_d = nc.dram_tensor("bias", [F_SH, 1], F32, kind="ExternalInput")
    y_d = nc.dram_tensor("y", [F_SH, B], OUT_DT, kind="ExternalOutput")

    PW = P // NWCH

    with TileContext(nc) as tc:
        with (
            tc.tile_pool(name="const", bufs=1) as const,
            tc.tile_pool(name="psum_r", bufs=2, space="PSUM") as psum_r,
            tc.tile_pool(name="psum_f", bufs=2, space="PSUM") as psum_f,
            tc.tile_pool(name="accs", bufs=ACC_BUFS) as accs,
            tc.tile_pool(name="rstage", bufs=3) as rstage,
            tc.tile_pool(name="stage", bufs=3) as stage,
            tc.tile_pool(name="outs", bufs=2) as outs,
        ):
            xt = const.tile([IDIM, B], MM_DT)
            wchunks = [
                const.tile([IDIM, PW, F_SH], MM_DT, name=f"wt{c}") for c in range(NWCH)
            ]
            nc.sync.dma_start(out=xt[:, 0:BCH], in_=xt_d[:, 0:BCH])
            nc.sync.dma_start(out=wchunks[0][:], in_=wt_d[:, 0:PW, :])
            nc.sync.dma_start(out=wchunks[1][:], in_=wt_d[:, PW : 2 * PW, :])
            ident = const.tile([F_SH, F_SH], STAGE_DT)
            nc.sync.dma_start(out=ident[:], in_=id_d[:])
            for c in range(1, NJ):
                nc.sync.dma_start(
                    out=xt[:, c * BCH : (c + 1) * BCH],
                    in_=xt_d[:, c * BCH : (c + 1) * BCH],
                )
            for c in range(2, NWCH):
                nc.sync.dma_start(
                    out=wchunks[c][:], in_=wt_d[:, c * PW : (c + 1) * PW, :]
                )
            sc = const.tile([F_SH, 1], F32)
            nc.sync.dma_start(out=sc[:], in_=sc_d[:])
            bi = const.tile([F_SH, 1], F32)
            nc.sync.dma_start(out=bi[:], in_=bi_d[:])
            warm = const.tile([F_SH, 2], F32)
            nc.vector.memset(warm[:], 0.0)
            nc.scalar.activation(
                out=warm[:, 1:2], in_=warm[:, 0:1],
                func=mybir.ActivationFunctionType.Relu,
            )

            def mm(pt, p, j, start=True, stop=True):
                nc.tensor.matmul(
                    pt,
                    wchunks[p // PW][:, p % PW, :],
                    xt[:, j * BCH : (j + 1) * BCH],
                    start=start,
                    stop=stop,
                )

            import contextlib

            loop_cm = (
                tc.For_i(0, REPS, 1, hint_engines=(mybir.EngineType.PE,))
                if REPS > 0
                else contextlib.nullcontext()
            )
            with loop_cm:
                acc_d = accs.tile([F_SH, NJ, GQ, BCH], STAGE_DT, tag="acc_d")
                acc_v = accs.tile([F_SH, NJ, GQ, BCH], STAGE_DT, tag="acc_v")
                n_d = n_v = 0
                fst_by_u = {}
                pending = []  # (unit, j, pt_f, rst) awaiting inject+drain

                def flush_pending():
                    nonlocal n_d, n_v
                    while pending:
                        u, j, pt_f, rst = pending.pop(0)
                        if rst is not None:  # R-unit: inject relu into bases
                            for q in range(GQ):
                                nc.tensor.matmul(
                                    pt_f[:, q, :],
                                    ident[:],
                                    rst[:, q, :],
                                    start=False,
                                    stop=True,
                                )
                        if drain[u] == "D":
                            dst = acc_d[:, j]
                            if n_d < NJ:
                                nc.vector.tensor_copy(out=dst, in_=pt_f[:])
                            else:
                                nc.vector.tensor_max(dst, pt_f[:], dst)
                            n_d += 1
                        else:
                            if u not in fst_by_u:
                                fst_by_u[u] = stage.tile(
                                    [F_SH, NJ, GQ, BCH], STAGE_DT,
                                    tag="fst", name="fst",
                                )
                            nc.scalar.activation(
                                out=fst_by_u[u][:, j],
                                in_=pt_f[:],
                                func=mybir.ActivationFunctionType.Copy,
                            )
                            if j == NJ - 1:
                                fst = fst_by_u.pop(u)
                                if n_v == 0:
                                    nc.vector.tensor_copy(out=acc_v[:], in_=fst[:])
                                else:
                                    nc.vector.tensor_max(acc_v[:], fst[:], acc_v[:])
                                n_v += 1

                # Bresenham-interleave D-drained and V-drained (unit, j)
                # blocks so DVE never starves during V-unit stretches.
                d_blocks = [
                    (u, j) for u in range(n_units) if drain[u] == "D"
                    for j in range(NJ)
                ]
                v_blocks = [
                    (u, j) for u in range(n_units) if drain[u] == "V"
                    for j in range(NJ)
                ]
                blocks = []
                di = vi = 0
                while di < len(d_blocks) or vi < len(v_blocks):
                    if vi * max(len(d_blocks), 1) <= di * max(len(v_blocks), 1):
                        if vi < len(v_blocks):
                            blocks.append(v_blocks[vi])
                            vi += 1
                        else:
                            blocks.append(d_blocks[di])
                            di += 1
                    elif di < len(d_blocks):
                        blocks.append(d_blocks[di])
                        di += 1
                    else:
                        blocks.append(v_blocks[vi])
                        vi += 1

                for u, j in blocks:
                    if True:
                        if u < UR:
                            # planes 4u..4u+3: slots 4u,4u+2 = diffs; 4u+1,4u+3 = bases
                            p0 = 4 * u
                            pt_r = psum_r.tile([F_SH, GQ, BCH], F32, tag="ptr", name="ptr")
                            mm(pt_r[:, 0, :], p0, j)
                            mm(pt_r[:, 1, :], p0 + 2, j)
                            rst = rstage.tile([F_SH, GQ, BCH], STAGE_DT, tag="rst", name="rst")
                            nc.scalar.activation(
                                out=rst[:],
                                in_=pt_r[:],
                                func=mybir.ActivationFunctionType.Relu,
                            )
                            pt_f = psum_f.tile([F_SH, GQ, BCH], F32, tag="ptf", name="ptf")
                            mm(pt_f[:, 0, :], p0 + 1, j, start=True, stop=False)
                            mm(pt_f[:, 1, :], p0 + 3, j, start=True, stop=False)
                            flush_pending()
                            pending.append((u, j, pt_f, rst))
                        else:
                            p0 = 4 * UR + 2 * (u - UR)
                            pt_f = psum_f.tile([F_SH, GQ, BCH], F32, tag="ptf", name="ptf")
                            mm(pt_f[:, 0, :], p0, j)
                            mm(pt_f[:, 1, :], p0 + 1, j)
                            flush_pending()
                            pending.append((u, j, pt_f, None))
                flush_pending()

                # ---- tails (once per body) -----------------------------
                def fold_gq(acc):
                    w = GQ
                    while w > 1:
                        h = w // 2
                        nc.vector.tensor_max(
                            acc[:, :, 0:h, :], acc[:, :, 0:h, :], acc[:, :, h : h + h, :]
                        )
                        w = h
                    return acc[:, :, 0, :]

                staged = fold_gq(acc_v) if n_v else None
                direct = fold_gq(acc_d) if n_d else None
                if staged is not None and direct is not None:
                    nc.vector.tensor_max(staged, direct, staged)
                    src = staged
                else:
                    src = staged if staged is not None else direct

                outt = outs.tile([F_SH, NJ, BCH], OUT_DT, tag="outt")
                if affine:
                    nc.vector.tensor_scalar(
                        out=outt[:],
                        in0=src,
                        scalar1=sc[:],
                        scalar2=bi[:],
                        op0=mybir.AluOpType.mult,
                        op1=mybir.AluOpType.add,
                    )
                    src = outt[:]
                elif src.dtype != OUT_DT:
                    nc.vector.tensor_copy(out=outt[:], in_=src)
                    src = outt[:]
                nc.sync.dma_start(out=y_d[:], in_=src)

    if fixup:
        split_multiwaits(nc)
    return nc


def build_nc_jint(assign=None, fixup=True, affine=True):
    """Whole-row variant with tile-level D/V interleaving.

    The two PSUM-drain engines (DVE for D-groups, ACT for V-groups) must run
    CONCURRENTLY -- a group-outer loop creates single-engine stretches that
    serialize them. So the (group, j) tile streams for D and V are
    Bresenham-merged into one emission order, with separate 2-buf PSUM pools
    per stream so each engine's fill/drain double-buffers independently.

    Accumulators span the full row (F_SH, NJ, GQ, BCH) so tail folds happen
    ONCE per kernel body; each staged V-group is folded by a single op on its
    assigned engine (POOL TT / SWDGE DMA-CCE / DVE TT)."""
    assign = assign or ASSIGN
    assign = assign.split(";")[0]
    assert len(assign) in (16, NG) and set(assign) <= set("DV")
    if len(assign) != NG:
        assign = "".join(c * (NG // 16) for c in assign)

    nc = bass.Bass()
    xt_d = nc.dram_tensor("xt", [IDIM, B], MM_DT, kind="ExternalInput")
    wt_d = nc.dram_tensor("wt", [IDIM, P, F_SH], MM_DT, kind="ExternalInput")
    sc_d = nc.dram_tensor("scale", [F_SH, 1], F32, kind="ExternalInput")
    bi_d = nc.dram_tensor("bias", [F_SH, 1], F32, kind="ExternalInput")
    y_d = nc.dram_tensor("y", [F_SH, B], OUT_DT, kind="ExternalOutput")

    PW = P // NWCH  # p-planes per weight chunk

    with TileContext(nc) as tc:
        with (
            tc.tile_pool(name="const", bufs=1) as const,
            tc.tile_pool(name="psum_d", bufs=2, space="PSUM") as psum_d,
            tc.tile_pool(name="psum_v", bufs=2, space="PSUM") as psum_v,
            tc.tile_pool(name="accs", bufs=ACC_BUFS) as accs,
            tc.tile_pool(name="stage", bufs=STAGE_BUFS) as stage,
            tc.tile_pool(name="outs", bufs=2) as outs,
        ):
            xt = const.tile([IDIM, B], MM_DT)
            wchunks = [
                const.tile([IDIM, PW, F_SH], MM_DT, name=f"wt{c}") for c in range(NWCH)
            ]
            nc.sync.dma_start(out=xt[:, 0:BCH], in_=xt_d[:, 0:BCH])
            nc.sync.dma_start(out=wchunks[0][:], in_=wt_d[:, 0:PW, :])
            nc.sync.dma_start(out=wchunks[1][:], in_=wt_d[:, PW : 2 * PW, :])
            for c in range(1, NJ):
                nc.sync.dma_start(
                    out=xt[:, c * BCH : (c + 1) * BCH],
                    in_=xt_d[:, c * BCH : (c + 1) * BCH],
                )
            for c in range(2, NWCH):
                nc.sync.dma_start(
                    out=wchunks[c][:], in_=wt_d[:, c * PW : (c + 1) * PW, :]
                )
            sc = const.tile([F_SH, 1], F32)
            nc.sync.dma_start(out=sc[:], in_=sc_d[:])
            bi = const.tile([F_SH, 1], F32)
            nc.sync.dma_start(out=bi[:], in_=bi_d[:])
            warm = const.tile([F_SH, 2], F32)
            nc.vector.memset(warm[:], 0.0)
            nc.scalar.activation(
                out=warm[:, 1:2], in_=warm[:, 0:1],
                func=mybir.ActivationFunctionType.Copy,
            )

            import contextlib

            loop_cm = (
                tc.For_i(0, REPS, 1, hint_engines=(mybir.EngineType.PE,))
                if REPS > 0
                else contextlib.nullcontext()
            )
            with loop_cm:
                acc_d = accs.tile([F_SH, NJ, GQ, BCH], STAGE_DT, tag="acc_d")
                acc_by = {}
                n_by = {"P": 0, "M": 0, "V": 0}
                for e in sorted(set(FOLD_PAT)):
                    acc_by[e] = accs.tile(
                        [F_SH, NJ, GQ, BCH], STAGE_DT, tag=f"acc_{e}", name=f"acc_{e}"
                    )
                n_d = 0
                fold_i = 0

                def eng_copy(eng, dst, src):
                    if eng == "V":
                        nc.vector.tensor_copy(out=dst, in_=src)
                    elif eng == "P":
                        nc.gpsimd.tensor_copy(out=dst, in_=src)
                    else:
                        nc.gpsimd.dma_start(out=dst, in_=src)

                def eng_max(eng, dst, src):
                    if eng == "V":
                        nc.vector.tensor_max(dst, src, dst)
                    elif eng == "P":
                        nc.gpsimd.tensor_max(dst, src, dst)
                    else:
                        nc.gpsimd.dma_start(out=dst, in_=src, accum_op=MX)

                # Bresenham-merge the D and V (group, j) tile streams so the
                # DVE and ACT drains interleave 1:1-ish and run concurrently.
                d_groups = [g for g in range(NG) if assign[g] == "D"]
                v_groups = [g for g in range(NG) if assign[g] == "V"]
                d_tiles = [(g, j) for g in d_groups for j in range(NJ)]
                v_tiles = [(g, j) for g in v_groups for j in range(NJ)]
                tiles = []
                di = vi = 0
                while di < len(d_tiles) or vi < len(v_tiles):
                    # emit whichever stream is proportionally behind
                    if vi * max(len(d_tiles), 1) <= di * max(len(v_tiles), 1):
                        if vi < len(v_tiles):
                            tiles.append(("V", *v_tiles[vi]))
                            vi += 1
                        else:
                            tiles.append(("D", *d_tiles[di]))
                            di += 1
                    elif di < len(d_tiles):
                        tiles.append(("D", *d_tiles[di]))
                        di += 1
                    else:
                        tiles.append(("V", *v_tiles[vi]))
                        vi += 1

                st_by_g = {}
                d_seen = set()
                for kind, g, j in tiles:
                    pool = psum_d if kind == "D" else psum_v
                    pt = pool.tile(
                        [F_SH, GQ, BCH], F32, tag=f"ps{kind}", name=f"pt{kind}"
                    )
                    for q in range(GQ):
                        p = GQ * g + q
                        nc.tensor.matmul(
                            pt[:, q, :],
                            wchunks[p // PW][:, p % PW, :],
                            xt[:, j * BCH : (j + 1) * BCH],
                            start=True,
                            stop=True,
                        )
                    if kind == "D":
                        dst = acc_d[:, j]
                        if g == d_groups[0]:
                            nc.vector.tensor_copy(out=dst, in_=pt[:])
                        else:
                            nc.vector.tensor_max(dst, pt[:], dst)
                    else:
                        if g not in st_by_g:
                            st_by_g[g] = stage.tile(
                                [F_SH, NJ, GQ, BCH], STAGE_DT, tag="st", name="st"
                            )
                        nc.scalar.activation(
                            out=st_by_g[g][:, j],
                            in_=pt[:],
                            func=mybir.ActivationFunctionType.Copy,
                        )
                        if j == NJ - 1:  # group fully staged -> fold it
                            st = st_by_g.pop(g)
                            eng = FOLD_PAT[fold_i % len(FOLD_PAT)]
                            fold_i += 1
                            if n_by[eng] == 0:
                                eng_copy(eng, acc_by[eng][:], st[:])
                            else:
                                eng_max(eng, acc_by[eng][:], st[:])
                            n_by[eng] += 1
                n_d = len(d_groups)

                # ---- tails (once per body) -----------------------------
                def fold_gq(acc):
                    # fold the GQ axis of (F_SH, NJ, GQ, BCH) down to slot 0
                    w = GQ
                    while w > 1:
                        h = w // 2
                        eng_max(
                            TAIL_ENG, acc[:, :, 0:h, :], acc[:, :, h : h + h, :]
                        )
                        w = h
                    return acc[:, :, 0, :]  # (F_SH, NJ, BCH)

                staged = None
                for e, n in n_by.items():
                    if n == 0:
                        continue
                    s = fold_gq(acc_by[e])
                    if staged is None:
                        staged = s
                    else:
                        eng_max(TAIL_ENG, staged, s)
                direct = fold_gq(acc_d) if n_d else None
                if staged is not None and direct is not None:
                    eng_max(TAIL_ENG, staged, direct)
                    src = staged
                else:
                    src = staged if staged is not None else direct

                outt = outs.tile([F_SH, NJ, BCH], OUT_DT, tag="outt")
                if affine:
                    nc.vector.tensor_scalar(
                        out=outt[:],
                        in0=src,
                        scalar1=sc[:],
                        scalar2=bi[:],
                        op0=mybir.AluOpType.mult,
                        op1=mybir.AluOpType.add,
                    )
                    src = outt[:]
                elif src.dtype != OUT_DT:
                    nc.vector.tensor_copy(out=outt[:], in_=src)
                    src = outt[:]
                nc.sync.dma_start(out=y_d[:], in_=src)

    if fixup:
        split_multiwaits(nc)
    return nc


def build_nc(assign=None, fixup=True, affine=True):
    if RELU:
        assert GQ == 2
        return build_nc_relu(fixup=fixup, affine=affine)
    if JINT:
        return build_nc_jint(assign=assign, fixup=fixup, affine=affine)
    assign = assign or ASSIGN
    pats = assign.split(";")
    if len(pats) == 1:
        pats = pats * NJ
    assert len(pats) == NJ
    expanded = []
    for p_ in pats:
        assert len(p_) in (16, NG) and set(p_) <= set("DV")
        if len(p_) != NG:
            p_ = "".join(c * (NG // 16) for c in p_)
        expanded.append(p_)
    pats = expanded

    nc = bass.Bass()
    xt_d = nc.dram_tensor("xt", [IDIM, B], MM_DT, kind="ExternalInput")
    wt_d = nc.dram_tensor("wt", [IDIM, P, F_SH], MM_DT, kind="ExternalInput")
    sc_d = nc.dram_tensor("scale", [F_SH, 1], F32, kind="ExternalInput")
    bi_d = nc.dram_tensor("bias", [F_SH, 1], F32, kind="ExternalInput")
    y_d = nc.dram_tensor("y", [F_SH, B], OUT_DT, kind="ExternalOutput")

    PW = P // NWCH  # p-planes per weight chunk
    VS = 2 * GQ  # staged-pair slot count (2 groups per staged tile)

    with TileContext(nc) as tc:
        with (
            tc.tile_pool(name="const", bufs=1) as const,
            tc.tile_pool(name="psum", bufs=PSUM_BUFS, space="PSUM") as psum,
            tc.tile_pool(name="accs", bufs=ACC_BUFS) as accs,
            tc.tile_pool(name="stage", bufs=STAGE_BUFS) as stage,
            tc.tile_pool(name="outs", bufs=2) as outs,
        ):
            # input loads: first-needed chunks first so group 0 starts ASAP
            xt = const.tile([IDIM, B], MM_DT)
            wchunks = [
                const.tile([IDIM, PW, F_SH], MM_DT, name=f"wt{c}") for c in range(NWCH)
            ]
            nc.sync.dma_start(out=xt[:, 0:BCH], in_=xt_d[:, 0:BCH])
            nc.sync.dma_start(out=wchunks[0][:], in_=wt_d[:, 0:PW, :])
            nc.sync.dma_start(out=wchunks[1][:], in_=wt_d[:, PW : 2 * PW, :])
            for c in range(2, NWCH):
                nc.sync.dma_start(
                    out=wchunks[c][:], in_=wt_d[:, c * PW : (c + 1) * PW, :]
                )
            for c in range(1, NJ):
                nc.sync.dma_start(
                    out=xt[:, c * BCH : (c + 1) * BCH],
                    in_=xt_d[:, c * BCH : (c + 1) * BCH],
                )
            sc = const.tile([F_SH, 1], F32)
            nc.sync.dma_start(out=sc[:], in_=sc_d[:])
            bi = const.tile([F_SH, 1], F32)
            nc.sync.dma_start(out=bi[:], in_=bi_d[:])
            warm = const.tile([F_SH, 2], F32)
            nc.vector.memset(warm[:], 0.0)
            nc.scalar.activation(
                out=warm[:, 1:2], in_=warm[:, 0:1],
                func=mybir.ActivationFunctionType.Copy,
            )

            import contextlib

            loop_cm = (
                tc.For_i(0, REPS, 1, hint_engines=(mybir.EngineType.PE,))
                if REPS > 0
                else contextlib.nullcontext()
            )
            with loop_cm:
              for j in range(NJ):
                assign_j = pats[j]
                rhs = xt[:, j * BCH : (j + 1) * BCH]

                acc_d = accs.tile([F_SH, GQ, BCH], STAGE_DT, tag="acc_d")
                # per-fold-engine staged accumulators (separate RMW chains so
                # POOL / DMA / DVE folds never serialize on one tensor)
                acc_by = {}
                n_by = {"P": 0, "M": 0, "V": 0}
                w_by = {}
                for e in set(FOLD_PAT + ODD_ENG):
                    acc_by[e] = accs.tile(
                        [F_SH, VS, BCH], STAGE_DT, tag=f"acc_{e}", name=f"acc_{e}"
                    )
                n_d = 0
                half = 0
                st = None
                fold_i = 0

                def eng_copy(eng, dst, src):
                    if eng == "V":
                        nc.vector.tensor_copy(out=dst, in_=src)
                    elif eng == "P":
                        nc.gpsimd.tensor_copy(out=dst, in_=src)
                    else:
                        # SWDGE DMA (Pool-dispatched, ~1us desc-gen + queue xfer)
                        nc.gpsimd.dma_start(out=dst, in_=src)

                def eng_max(eng, dst, src):
                    if eng == "V":
                        nc.vector.tensor_max(dst, src, dst)
                    elif eng == "P":
                        nc.gpsimd.tensor_max(dst, src, dst)
                    else:
                        nc.gpsimd.dma_start(out=dst, in_=src, accum_op=MX)

                def flush_pair(full):
                    nonlocal st, fold_i
                    eng = FOLD_PAT[fold_i % len(FOLD_PAT)] if full else ODD_ENG
                    if full:
                        fold_i += 1
                    acc = acc_by[eng]
                    n = n_by[eng]
                    if full:
                        src = st[:].rearrange("p a g b -> p (a g) b")
                        dst = acc[:]
                    else:
                        src = st[:, 0]
                        dst = acc[:, 0:GQ, :]
                    if n == 0:
                        eng_copy(eng, dst, src)
                        w_by[eng] = VS if full else GQ
                    else:
                        eng_max(eng, dst, src)
                        w_by[eng] = max(w_by[eng], VS if full else GQ)
                    n_by[eng] = n + 1
                    st = None

                for g in range(NG):
                    kind = assign_j[g]
                    pt = psum.tile([F_SH, GQ, BCH], F32, tag="ps")
                    for q in range(GQ):
                        p = GQ * g + q
                        nc.tensor.matmul(
                            pt[:, q, :],
                            wchunks[p // PW][:, p % PW, :],
                            rhs,
                            start=True,
                            stop=True,
                        )
                    if kind == "D":
                        if n_d == 0:
                            nc.vector.tensor_copy(out=acc_d[:], in_=pt[:])
                        else:
                            nc.vector.tensor_max(acc_d[:], pt[:], acc_d[:])
                        n_d += 1
                    else:
                        if st is None:
                            st = stage.tile([F_SH, 2, GQ, BCH], STAGE_DT, tag="st")
                        nc.scalar.activation(
                            out=st[:, half],
                            in_=pt[:],
                            func=mybir.ActivationFunctionType.Copy,
                        )
                        half ^= 1
                        if half == 0:
                            flush_pair(full=True)
                if half == 1:
                    flush_pair(full=False)

                # ---- tails ------------------------------------------------
                def fold_chain(acc, w):
                    # fold acc[:, 0:w, :] down to acc[:, 0:1, :]
                    while w > 1:
                        h = w // 2
                        eng_max(TAIL_ENG, acc[:, 0:h, :], acc[:, h : h + h, :])
                        if w % 2 == 1:  # odd leftover plane
                            eng_max(TAIL_ENG, acc[:, 0:1, :], acc[:, w - 1 : w, :])
                        w = h
                    return acc[:, 0, :]

                staged = None
                for e, n in n_by.items():
                    if n == 0:
                        continue
                    s = fold_chain(acc_by[e], w_by[e])
                    if staged is None:
                        staged = s
                    else:
                        eng_max(TAIL_ENG, staged, s)
                direct = None
                if n_d:
                    direct = fold_chain(acc_d, GQ)
                if staged is not None and direct is not None:
                    eng_max(TAIL_ENG, staged, direct)
                    src = staged
                else:
                    src = staged if staged is not None else direct

                outt = outs.tile([F_SH, BCH], OUT_DT, tag="outt")
                if affine:
                    nc.vector.tensor_scalar(
                        out=outt[:],
                        in0=src,
                        scalar1=sc[:],
                        scalar2=bi[:],
                        op0=mybir.AluOpType.mult,
                        op1=mybir.AluOpType.add,
                    )
                    src = outt[:]
                elif src.dtype != OUT_DT:
                    nc.vector.tensor_copy(out=outt[:], in_=src)
                    src = outt[:]
                nc.sync.dma_start(out=y_d[:, j * BCH : (j + 1) * BCH], in_=src)

    if fixup:
        split_multiwaits(nc)
    return nc


_CACHED_NC = None


def _get_nc():
    global _CACHED_NC
    if _CACHED_NC is None:
        _CACHED_NC = build_nc()
    return _CACHED_NC


def make_in_maps(x, ww, scale, bias):
    x = np.asarray(x)
    ww = np.asarray(ww)
    scale = np.asarray(scale)
    bias = np.asarray(bias)

    xf = np.ascontiguousarray(x.reshape(B, IDIM).T).astype(np.float32)  # (64, 2048)
    wwf = ww.reshape(FDIM, P, IDIM)
    sc = scale.reshape(FDIM).astype(np.float32)
    bi = bias.reshape(FDIM).astype(np.float32)

    in_maps = []
    for k in range(N_CORES):
        wk = wwf[k * F_SH : (k + 1) * F_SH]  # (128, 64, 64) = (f, p, i)
        wt = np.ascontiguousarray(wk.transpose(2, 1, 0)).astype(np.float32)  # (i,p,f)
        if RELU:
            # R-unit weight slots: for each relu-pair (2p, 2p+1) with
            # p-pairs packed 4 planes per unit: slot 4u   = w[4u]   - w[4u+1]
            #                                   slot 4u+1 = w[4u+1]
            #                                   slot 4u+2 = w[4u+2] - w[4u+3]
            #                                   slot 4u+3 = w[4u+3]
            wt = wt.copy()
            for u in range(UR):
                p0 = 4 * u
                wt[:, p0, :] = wt[:, p0, :] - wt[:, p0 + 1, :]
                wt[:, p0 + 2, :] = wt[:, p0 + 2, :] - wt[:, p0 + 3, :]
        m = {
            "xt": xf,
            "wt": np.ascontiguousarray(wt),
            "scale": np.ascontiguousarray(
                sc[k * F_SH : (k + 1) * F_SH].reshape(F_SH, 1)
            ),
            "bias": np.ascontiguousarray(
                bi[k * F_SH : (k + 1) * F_SH].reshape(F_SH, 1)
            ),
        }
        if RELU:
            m["ident"] = np.eye(F_SH, dtype=mybir.dt.np(STAGE_DT))
        in_maps.append(m)
    return in_maps


def kernel(x, ww, scale, bias):
    in_maps = make_in_maps(x, ww, scale, bias)
    trivial_affine = bool(
        np.all(np.asarray(scale) == 1.0) and np.all(np.asarray(bias) == 0.0)
    )
    nc = build_nc(affine=not trivial_affine)
    res = run_bass_kernel_spmd(nc, in_maps, list(range(N_CORES)))
    out = np.empty((FDIM, B), dtype=np.float32)
    for k in range(N_CORES):
        out[k * F_SH : (k + 1) * F_SH] = res.results[k]["y"].astype(np.float32)
    return np.ascontiguousarray(out.T)


# revision 21
# speedup vs baseline: 1.7247x; 1.5598x over previous
"""Trainium2 Bass kernel for nn_AffineLayer (topk_masking):
out[b, f] = max_p(x[b] . ww[f, p]) * scale[f] + bias[f]

Shapes (hardcoded per problem spec):
  x     (2048, 1, 8, 8)  -> xf (2048, 64)
  ww    (1024, 64, 1, 8, 8) -> wwf (1024, 64, 64)   (f, p, i)
  scale (1, 1024), bias (1, 1024)
  out   (2048, 1024)

Sharding: f tensor-parallel over 8 cores (f_shard = 128 per core), x replicated.

Per-core device layout (f on partitions):
  lhsT (stationary) = wT[:, p, :] : (i=64, f=128)  per p-plane
  rhs  (moving)     = xT[:, bchunk]: (i=64, b=512)
  psum out          = (f=128, b=512) per p-plane, 1 PSUM bank

The 64-way max over p is the bottleneck: every score must leave PSUM through
one of the only two PSUM-capable engines (DVE and ACT, both 1 elem/cycle/lane
for fp32). p-plane groups are split between:
  - DVE: running tensor_tensor(max) straight from PSUM into a 4-slot fp32 acc
  - ACT: activation(Copy) PSUM -> SBUF staging (cast to fp16: same 16-bit
    2x fold speed as bf16, 8x the mantissa precision), folded into 16-bit
    accumulators by DVE tensor_tensor at 2x packed rate.
Final per-chunk: fold acc slots, combine paths, apply scale/bias via one
tensor_scalar with per-partition (f) scalars, DMA out as (128f, 2048b).
Host reassembles and transposes to (2048, 1024).

(Explored this session and rejected on real-HW grounds: POOL tensor-tensor
folds -- no such opcode in V3 codegen; SDMA CCE accum-max folds -- verifier
rejects max in Copy mode; 16-bit PSUM for 2x DVE drains -- TRN3-only;
relu-pair compression via PE identity re-injection -- numerically correct
but PE clock-gating (1.2 GHz without ~4us continuous busy) makes the extra
PE work bind, measured ~205us vs the ~125us baseline.)
"""

import os
import sys

if "/opt/trn_rl_repo" not in sys.path:
    sys.path.insert(0, "/opt/trn_rl_repo")

import numpy as np

import concourse.bass as bass
import concourse.mybir as mybir
from concourse.tile import TileContext
from concourse.bass_utils import run_bass_kernel_spmd

# Problem dims (hardcoded)
B, FDIM, P, IDIM = 2048, 1024, 64, 64
N_CORES = 8
F_SH = FDIM // N_CORES  # 128
BCH = 512  # b-chunk size (PSUM bank = 512 fp32)
NJ = B // BCH  # 4
GQ = int(os.environ.get("KGQ", "2"))  # p-planes per group (= PSUM banks)
NG = P // GQ  # groups
PSUM_BUFS = 8 // GQ

# ---- Tunables ----------------------------------------------------------
# Per-group drain assignment, length NG. "D" = DVE direct TT-max from PSUM;
# "V" = ACT copy -> staged, folded by DVE.
ASSIGN = os.environ.get("KASSIGN", "VVDVVVDVVVDVVVDVVVDVVVDVVVDVVVDV")
STAGE_BF16 = os.environ.get("KSTAGE_BF16", "1") == "1"
# Staging dtype: fp16 matches bf16's 2x DVE fold speed (both 16-bit) but has
# 10 mantissa bits vs 7 -- scores (|s| < ~70) sit far inside fp16 range.
STAGE_DT_NAME = os.environ.get("KSTAGE_DT", "float16" if STAGE_BF16 else "float32")
# Matmul input dtype: float32r streams 1 row/cycle on the PE (vs 4 for fp32,
# which decomposes into 2 half-speed passes); same 4-byte layout as fp32.
MM_DT_NAME = os.environ.get("KMM_DT", "float32r")
NWCH = int(os.environ.get("KNWCH", "32"))
REPS = int(os.environ.get("KREPS", "0"))  # >0: wrap body in a For_i repeat loop (bench only)
STAGE_BUFS = int(os.environ.get("KSTAGE_BUFS", "6"))
DQUAD = os.environ.get("KDQUAD", "0") == "1"  # D-groups drain as 4-bank quads
OUT16 = os.environ.get("KOUT16", "0") == "1"  # store y in STAGE_DT, upconvert on host
# ------------------------------------------------------------------------

F32 = mybir.dt.float32
BF16 = mybir.dt.bfloat16
STAGE_DT = getattr(mybir.dt, STAGE_DT_NAME)
MM_DT = getattr(mybir.dt, MM_DT_NAME)
OUT_DT = STAGE_DT if OUT16 else F32
MX = mybir.AluOpType.max


def split_multiwaits(nc):
    """This walrus build allows at most ONE sem wait per instruction.
    Tile's wait assignment can emit several; hoist extras onto inserted
    sequencer nops immediately before the over-subscribed instruction
    (same engine, program order preserved => identical semantics)."""
    wid = 0
    for f in nc.m.functions:
        for bb in f.blocks:
            il = bb.instructions
            i = 0
            while i < len(il):
                ins = il[i]
                si = getattr(ins, "sync_info", None)
                if si is not None and si.on_wait and len(si.on_wait) > 1:
                    waits = list(si.on_wait)
                    si.on_wait = waits[-1:]
                    carriers = []
                    for w in waits[:-1]:
                        wid += 1
                        carriers.append(
                            mybir.InstNoOp(
                                name=f"WSPLIT-{wid}",
                                engine=ins.engine,
                                sync_info=mybir.SyncInfo(on_wait=[w], on_update=[]),
                            )
                        )
                    il[i:i] = carriers
                    i += len(carriers)
                i += 1


def build_nc(assign=None, fixup=True, affine=True):
    assign = assign or ASSIGN
    pats = assign.split(";")
    if len(pats) == 1:
        pats = pats * NJ
    assert len(pats) == NJ
    expanded = []
    for p_ in pats:
        assert len(p_) in (16, NG) and set(p_) <= set("DV")
        if len(p_) != NG:
            p_ = "".join(c * (NG // 16) for c in p_)
        expanded.append(p_)
    pats = expanded

    nc = bass.Bass()
    xt_d = nc.dram_tensor("xt", [IDIM, B], MM_DT, kind="ExternalInput")
    wt_d = nc.dram_tensor("wt", [IDIM, P, F_SH], MM_DT, kind="ExternalInput")
    sc_d = nc.dram_tensor("scale", [F_SH, 1], F32, kind="ExternalInput")
    bi_d = nc.dram_tensor("bias", [F_SH, 1], F32, kind="ExternalInput")
    y_d = nc.dram_tensor("y", [F_SH, B], OUT_DT, kind="ExternalOutput")

    PW = P // NWCH  # p-planes per weight chunk
    VS = 2 * GQ  # staged-pair slot count (2 groups per staged tile)

    with TileContext(nc) as tc:
        with (
            tc.tile_pool(name="const", bufs=1) as const,
            tc.tile_pool(name="psum", bufs=PSUM_BUFS, space="PSUM") as psum,
            tc.tile_pool(
                name="accs", bufs=int(os.environ.get("KACC_BUFS", "2"))
            ) as accs,
            tc.tile_pool(name="stage", bufs=STAGE_BUFS) as stage,
            tc.tile_pool(
                name="outs", bufs=int(os.environ.get("KOUT_BUFS", "2"))
            ) as outs,
        ):
            # input loads: first-needed chunks first so group 0 starts ASAP
            xt = const.tile([IDIM, B], MM_DT)
            wchunks = [
                const.tile([IDIM, PW, F_SH], MM_DT, name=f"wt{c}") for c in range(NWCH)
            ]
            nc.sync.dma_start(out=xt[:, 0:BCH], in_=xt_d[:, 0:BCH])
            nc.sync.dma_start(out=wchunks[0][:], in_=wt_d[:, 0:PW, :])
            nc.sync.dma_start(out=wchunks[1][:], in_=wt_d[:, PW : 2 * PW, :])
            for c in range(2, NWCH):
                nc.sync.dma_start(
                    out=wchunks[c][:], in_=wt_d[:, c * PW : (c + 1) * PW, :]
                )
            for c in range(1, NJ):
                nc.sync.dma_start(
                    out=xt[:, c * BCH : (c + 1) * BCH],
                    in_=xt_d[:, c * BCH : (c + 1) * BCH],
                )
            sc = const.tile([F_SH, 1], F32)
            nc.sync.dma_start(out=sc[:], in_=sc_d[:])
            bi = const.tile([F_SH, 1], F32)
            nc.sync.dma_start(out=bi[:], in_=bi_d[:])
            warm = const.tile([F_SH, 2], F32)
            nc.vector.memset(warm[:], 0.0)
            nc.scalar.activation(
                out=warm[:, 1:2], in_=warm[:, 0:1],
                func=mybir.ActivationFunctionType.Copy,
            )

            import contextlib

            loop_cm = (
                tc.For_i(0, REPS, 1, hint_engines=(mybir.EngineType.PE,))
                if REPS > 0
                else contextlib.nullcontext()
            )
            with loop_cm:
              for j in range(NJ):
                assign_j = pats[j]
                last_d = assign_j.rfind("D")
                rhs = xt[:, j * BCH : (j + 1) * BCH]
                DS = 4 if DQUAD else GQ
                acc_d = accs.tile([F_SH, DS, BCH], F32, tag="acc_d")
                acc_v = accs.tile([F_SH, VS, BCH], STAGE_DT, tag="acc_v")
                n_d = n_v = 0
                half = 0  # staged-pair fill state
                st = None

                def flush_pair(full):
                    nonlocal n_v, st
                    if full:
                        src = st[:].rearrange("p a g b -> p (a g) b")
                        dst = acc_v[:]
                    else:
                        src = st[:, 0]
                        dst = acc_v[:, 0:GQ, :]
                    if n_v == 0:
                        nc.vector.tensor_copy(out=dst, in_=src)
                    else:
                        nc.vector.tensor_max(dst, src, dst)
                    n_v += 1
                    st = None

                # build token schedule: D-pairs become 4-bank quads in DQUAD mode
                tokens = []
                g = 0
                while g < NG:
                    if (
                        DQUAD
                        and assign_j[g] == "D"
                    ):
                        assert g + 1 < NG and assign_j[g + 1] == "D", (
                            "KDQUAD=1 requires D groups in adjacent pairs"
                        )
                        tokens.append(("D", g, 2 * GQ))
                        g += 2
                    else:
                        tokens.append((assign_j[g], g, GQ))
                        g += 1
                n_dtok = sum(1 for t in tokens if t[0] == "D")
                dtok_i = 0
                for kind, g0, nplanes in tokens:
                    if kind == "D" and DQUAD:
                        pt = psum.tile([F_SH, 4, BCH], F32, tag="psd", bufs=1, name="ptd")
                    else:
                        pt = psum.tile(
                            [F_SH, GQ, BCH],
                            F32,
                            tag="ps",
                            bufs=2 if DQUAD else PSUM_BUFS,
                            name="ptv",
                        )
                    for q in range(nplanes):
                        p = GQ * g0 + q
                        nc.tensor.matmul(
                            pt[:, q, :],
                            wchunks[p // PW][:, p % PW, :],
                            rhs,
                            start=True,
                            stop=True,
                        )
                    if kind == "D":
                        dst = acc_d[:] if nplanes == DS else acc_d[:, 0:nplanes, :]
                        if n_d == 0:
                            assert nplanes == DS, "first D token must fill acc_d"
                            nc.vector.tensor_copy(out=dst, in_=pt[:])
                        else:
                            nc.vector.tensor_max(dst, pt[:], dst)
                        n_d += 1
                        dtok_i += 1
                        if dtok_i == n_dtok:
                            w = DS
                            while w > 1:
                                h = w // 2
                                nc.vector.tensor_max(
                                    acc_d[:, 0:h, :],
                                    acc_d[:, 0:h, :],
                                    acc_d[:, h:w, :],
                                )
                                w = h
                    else:
                        if st is None:
                            st = stage.tile([F_SH, 2, GQ, BCH], STAGE_DT, tag="st")
                        nc.scalar.activation(
                            out=st[:, half],
                            in_=pt[:],
                            func=mybir.ActivationFunctionType.Copy,
                        )
                        half ^= 1
                        if half == 0:
                            flush_pair(full=True)
                if half == 1:
                    flush_pair(full=False)

                # ---- tails ------------------------------------------------
                staged = None
                if n_v:
                    w = VS
                    while w > 1:
                        h = w // 2
                        nc.vector.tensor_max(
                            acc_v[:, 0:h, :], acc_v[:, 0:h, :], acc_v[:, h:w, :]
                        )
                        w = h
                    staged = acc_v[:, 0, :]
                direct = acc_d[:, 0, :] if n_d else None

                outt = outs.tile([F_SH, BCH], OUT_DT, tag="outt")
                if direct is not None and staged is not None:
                    nc.vector.tensor_max(outt[:], direct, staged)  # mixed dtype OK
                    src = outt[:]
                elif direct is not None:
                    src = direct
                else:
                    src = staged
                if affine:
                    nc.vector.tensor_scalar(
                        out=outt[:],
                        in0=src,
                        scalar1=sc[:],
                        scalar2=bi[:],
                        op0=mybir.AluOpType.mult,
                        op1=mybir.AluOpType.add,
                    )
                    src = outt[:]
                elif src is not outt[:] and src.dtype != OUT_DT:
                    nc.vector.tensor_copy(out=outt[:], in_=src)
                    src = outt[:]
                nc.sync.dma_start(out=y_d[:, j * BCH : (j + 1) * BCH], in_=src)

    if fixup:
        split_multiwaits(nc)
    return nc


_CACHED_NC = None


def _get_nc():
    global _CACHED_NC
    if _CACHED_NC is None:
        _CACHED_NC = build_nc()
    return _CACHED_NC


def make_in_maps(x, ww, scale, bias):
    x = np.asarray(x)
    ww = np.asarray(ww)
    scale = np.asarray(scale)
    bias = np.asarray(bias)

    xf = np.ascontiguousarray(x.reshape(B, IDIM).T).astype(np.float32)  # (64, 2048)
    wwf = ww.reshape(FDIM, P, IDIM)
    sc = scale.reshape(FDIM).astype(np.float32)
    bi = bias.reshape(FDIM).astype(np.float32)

    in_maps = []
    for k in range(N_CORES):
        wk = wwf[k * F_SH : (k + 1) * F_SH]  # (128, 64, 64) = (f, p, i)
        wt = np.ascontiguousarray(wk.transpose(2, 1, 0)).astype(np.float32)  # (i,p,f)
        in_maps.append(
            {
                "xt": xf,
                "wt": wt,
                "scale": np.ascontiguousarray(
                    sc[k * F_SH : (k + 1) * F_SH].reshape(F_SH, 1)
                ),
                "bias": np.ascontiguousarray(
                    bi[k * F_SH : (k + 1) * F_SH].reshape(F_SH, 1)
                ),
            }
        )
    return in_maps


def kernel(x, ww, scale, bias):
    in_maps = make_in_maps(x, ww, scale, bias)
    trivial_affine = bool(
        np.all(np.asarray(scale) == 1.0) and np.all(np.asarray(bias) == 0.0)
    )
    nc = build_nc(affine=not trivial_affine)
    res = run_bass_kernel_spmd(nc, in_maps, list(range(N_CORES)))
    out = np.empty((FDIM, B), dtype=np.float32)
    for k in range(N_CORES):
        out[k * F_SH : (k + 1) * F_SH] = res.results[k]["y"].astype(np.float32)
    return np.ascontiguousarray(out.T)


# revision 22
# speedup vs baseline: 1.7540x; 1.0170x over previous
"""Trainium2 Bass kernel for nn_AffineLayer (topk_masking):
out[b, f] = max_p(x[b] . ww[f, p]) * scale[f] + bias[f]

Shapes (hardcoded per problem spec):
  x     (2048, 1, 8, 8)  -> xf (2048, 64)
  ww    (1024, 64, 1, 8, 8) -> wwf (1024, 64, 64)   (f, p, i)
  scale (1, 1024), bias (1, 1024)
  out   (2048, 1024)

Sharding: f tensor-parallel over 8 cores (f_shard = 128 per core), x replicated.

Per-core device layout (f on partitions):
  lhsT (stationary) = wT[:, p, :] : (i=64, f=128)  per p-plane
  rhs  (moving)     = xT[:, bchunk]: (i=64, b=512)
  psum out          = (f=128, b=512) per p-plane, 1 PSUM bank

The 64-way max over p is the bottleneck: every score must leave PSUM through
one of the only two PSUM-capable engines (DVE and ACT, both 1 elem/cycle/lane
for fp32). p-plane groups are split between:
  - DVE: running tensor_tensor(max) straight from PSUM into a 4-slot fp32 acc
  - ACT: activation(Copy) PSUM -> SBUF staging (cast to fp16: same 16-bit
    2x fold speed as bf16, 8x the mantissa precision), folded into 16-bit
    accumulators by DVE tensor_tensor at 2x packed rate.
Final per-chunk: fold acc slots, combine paths, apply scale/bias via one
tensor_scalar with per-partition (f) scalars, DMA out as (128f, 2048b).
Host reassembles and transposes to (2048, 1024).

(Explored this session and rejected on real-HW grounds: POOL tensor-tensor
folds -- no such opcode in V3 codegen; SDMA CCE accum-max folds -- verifier
rejects max in Copy mode; 16-bit PSUM for 2x DVE drains -- TRN3-only;
relu-pair compression via PE identity re-injection -- numerically correct
but PE clock-gating (1.2 GHz without ~4us continuous busy) makes the extra
PE work bind, measured ~205us vs the ~125us baseline.)
"""

import os
import sys

if "/opt/trn_rl_repo" not in sys.path:
    sys.path.insert(0, "/opt/trn_rl_repo")

import numpy as np

import concourse.bass as bass
import concourse.mybir as mybir
from concourse.tile import TileContext
from concourse.bass_utils import run_bass_kernel_spmd

# Problem dims (hardcoded)
B, FDIM, P, IDIM = 2048, 1024, 64, 64
N_CORES = 8
F_SH = FDIM // N_CORES  # 128
BCH = 512  # b-chunk size (PSUM bank = 512 fp32)
NJ = B // BCH  # 4
GQ = int(os.environ.get("KGQ", "2"))  # p-planes per group (= PSUM banks)
NG = P // GQ  # groups
PSUM_BUFS = 8 // GQ

# ---- Tunables ----------------------------------------------------------
# Per-group drain assignment, length NG. "D" = DVE direct TT-max from PSUM;
# "V" = ACT copy -> staged, folded by DVE.
ASSIGN = os.environ.get("KASSIGN", "VVDVVVDVVVDVVVDVVVDVVVDVVVDVVVDV")
STAGE_BF16 = os.environ.get("KSTAGE_BF16", "1") == "1"
# Staging dtype: fp16 matches bf16's 2x DVE fold speed (both 16-bit) but has
# 10 mantissa bits vs 7 -- scores (|s| < ~70) sit far inside fp16 range.
STAGE_DT_NAME = os.environ.get("KSTAGE_DT", "float16" if STAGE_BF16 else "float32")
# Matmul input dtype: float32r streams 1 row/cycle on the PE (vs 4 for fp32,
# which decomposes into 2 half-speed passes); same 4-byte layout as fp32.
MM_DT_NAME = os.environ.get("KMM_DT", "float32r")
NWCH = int(os.environ.get("KNWCH", "32"))
REPS = int(os.environ.get("KREPS", "0"))  # >0: wrap body in a For_i repeat loop (bench only)
STAGE_BUFS = int(os.environ.get("KSTAGE_BUFS", "6"))
DQUAD = os.environ.get("KDQUAD", "0") == "1"  # D-groups drain as 4-bank quads
# Store y in fp16: halves the output-store DMA, lets the affine tensor_scalar
# hit DVE 4x mode (all-SBUF 2-byte operands), host upconverts. Adds ~2e-4 of
# fp16 rounding on top of the staged path's existing fp16 error -- well under
# the 2e-2 gate.
OUT16 = os.environ.get("KOUT16", "1") == "1"
# ------------------------------------------------------------------------

F32 = mybir.dt.float32
BF16 = mybir.dt.bfloat16
STAGE_DT = getattr(mybir.dt, STAGE_DT_NAME)
MM_DT = getattr(mybir.dt, MM_DT_NAME)
OUT_DT = STAGE_DT if OUT16 else F32
MX = mybir.AluOpType.max


def split_multiwaits(nc):
    """This walrus build allows at most ONE sem wait per instruction.
    Tile's wait assignment can emit several; hoist extras onto inserted
    sequencer nops immediately before the over-subscribed instruction
    (same engine, program order preserved => identical semantics)."""
    wid = 0
    for f in nc.m.functions:
        for bb in f.blocks:
            il = bb.instructions
            i = 0
            while i < len(il):
                ins = il[i]
                si = getattr(ins, "sync_info", None)
                if si is not None and si.on_wait and len(si.on_wait) > 1:
                    waits = list(si.on_wait)
                    si.on_wait = waits[-1:]
                    carriers = []
                    for w in waits[:-1]:
                        wid += 1
                        carriers.append(
                            mybir.InstNoOp(
                                name=f"WSPLIT-{wid}",
                                engine=ins.engine,
                                sync_info=mybir.SyncInfo(on_wait=[w], on_update=[]),
                            )
                        )
                    il[i:i] = carriers
                    i += len(carriers)
                i += 1


def build_nc(assign=None, fixup=True, affine=True):
    assign = assign or ASSIGN
    pats = assign.split(";")
    if len(pats) == 1:
        pats = pats * NJ
    assert len(pats) == NJ
    expanded = []
    for p_ in pats:
        assert len(p_) in (16, NG) and set(p_) <= set("DV")
        if len(p_) != NG:
            p_ = "".join(c * (NG // 16) for c in p_)
        expanded.append(p_)
    pats = expanded

    nc = bass.Bass()
    xt_d = nc.dram_tensor("xt", [IDIM, B], MM_DT, kind="ExternalInput")
    wt_d = nc.dram_tensor("wt", [IDIM, P, F_SH], MM_DT, kind="ExternalInput")
    sc_d = nc.dram_tensor("scale", [F_SH, 1], F32, kind="ExternalInput")
    bi_d = nc.dram_tensor("bias", [F_SH, 1], F32, kind="ExternalInput")
    y_d = nc.dram_tensor("y", [F_SH, B], OUT_DT, kind="ExternalOutput")

    PW = P // NWCH  # p-planes per weight chunk
    VS = 2 * GQ  # staged-pair slot count (2 groups per staged tile)

    with TileContext(nc) as tc:
        with (
            tc.tile_pool(name="const", bufs=1) as const,
            tc.tile_pool(name="psum", bufs=PSUM_BUFS, space="PSUM") as psum,
            tc.tile_pool(
                name="accs", bufs=int(os.environ.get("KACC_BUFS", "2"))
            ) as accs,
            tc.tile_pool(name="stage", bufs=STAGE_BUFS) as stage,
            tc.tile_pool(
                name="outs", bufs=int(os.environ.get("KOUT_BUFS", "2"))
            ) as outs,
        ):
            # input loads: first-needed chunks first so group 0 starts ASAP
            xt = const.tile([IDIM, B], MM_DT)
            wchunks = [
                const.tile([IDIM, PW, F_SH], MM_DT, name=f"wt{c}") for c in range(NWCH)
            ]
            nc.sync.dma_start(out=xt[:, 0:BCH], in_=xt_d[:, 0:BCH])
            nc.sync.dma_start(out=wchunks[0][:], in_=wt_d[:, 0:PW, :])
            nc.sync.dma_start(out=wchunks[1][:], in_=wt_d[:, PW : 2 * PW, :])
            for c in range(2, NWCH):
                nc.sync.dma_start(
                    out=wchunks[c][:], in_=wt_d[:, c * PW : (c + 1) * PW, :]
                )
            for c in range(1, NJ):
                nc.sync.dma_start(
                    out=xt[:, c * BCH : (c + 1) * BCH],
                    in_=xt_d[:, c * BCH : (c + 1) * BCH],
                )
            sc = const.tile([F_SH, 1], F32)
            nc.sync.dma_start(out=sc[:], in_=sc_d[:])
            bi = const.tile([F_SH, 1], F32)
            nc.sync.dma_start(out=bi[:], in_=bi_d[:])
            warm = const.tile([F_SH, 2], F32)
            nc.vector.memset(warm[:], 0.0)
            nc.scalar.activation(
                out=warm[:, 1:2], in_=warm[:, 0:1],
                func=mybir.ActivationFunctionType.Copy,
            )

            import contextlib

            loop_cm = (
                tc.For_i(0, REPS, 1, hint_engines=(mybir.EngineType.PE,))
                if REPS > 0
                else contextlib.nullcontext()
            )
            with loop_cm:
              for j in range(NJ):
                assign_j = pats[j]
                last_d = assign_j.rfind("D")
                rhs = xt[:, j * BCH : (j + 1) * BCH]
                DS = 4 if DQUAD else GQ
                acc_d = accs.tile([F_SH, DS, BCH], F32, tag="acc_d")
                acc_v = accs.tile([F_SH, VS, BCH], STAGE_DT, tag="acc_v")
                n_d = n_v = 0
                half = 0  # staged-pair fill state
                st = None

                def flush_pair(full):
                    nonlocal n_v, st
                    if full:
                        src = st[:].rearrange("p a g b -> p (a g) b")
                        dst = acc_v[:]
                    else:
                        src = st[:, 0]
                        dst = acc_v[:, 0:GQ, :]
                    if n_v == 0:
                        nc.vector.tensor_copy(out=dst, in_=src)
                    else:
                        nc.vector.tensor_max(dst, src, dst)
                    n_v += 1
                    st = None

                # build token schedule: D-pairs become 4-bank quads in DQUAD mode
                tokens = []
                g = 0
                while g < NG:
                    if (
                        DQUAD
                        and assign_j[g] == "D"
                    ):
                        assert g + 1 < NG and assign_j[g + 1] == "D", (
                            "KDQUAD=1 requires D groups in adjacent pairs"
                        )
                        tokens.append(("D", g, 2 * GQ))
                        g += 2
                    else:
                        tokens.append((assign_j[g], g, GQ))
                        g += 1
                n_dtok = sum(1 for t in tokens if t[0] == "D")
                dtok_i = 0
                for kind, g0, nplanes in tokens:
                    if kind == "D" and DQUAD:
                        pt = psum.tile([F_SH, 4, BCH], F32, tag="psd", bufs=1, name="ptd")
                    else:
                        pt = psum.tile(
                            [F_SH, GQ, BCH],
                            F32,
                            tag="ps",
                            bufs=2 if DQUAD else PSUM_BUFS,
                            name="ptv",
                        )
                    for q in range(nplanes):
                        p = GQ * g0 + q
                        nc.tensor.matmul(
                            pt[:, q, :],
                            wchunks[p // PW][:, p % PW, :],
                            rhs,
                            start=True,
                            stop=True,
                        )
                    if kind == "D":
                        dst = acc_d[:] if nplanes == DS else acc_d[:, 0:nplanes, :]
                        if n_d == 0:
                            assert nplanes == DS, "first D token must fill acc_d"
                            nc.vector.tensor_copy(out=dst, in_=pt[:])
                        else:
                            nc.vector.tensor_max(dst, pt[:], dst)
                        n_d += 1
                        dtok_i += 1
                        if dtok_i == n_dtok:
                            w = DS
                            while w > 1:
                                h = w // 2
                                nc.vector.tensor_max(
                                    acc_d[:, 0:h, :],
                                    acc_d[:, 0:h, :],
                                    acc_d[:, h:w, :],
                                )
                                w = h
                    else:
                        if st is None:
                            st = stage.tile([F_SH, 2, GQ, BCH], STAGE_DT, tag="st")
                        nc.scalar.activation(
                            out=st[:, half],
                            in_=pt[:],
                            func=mybir.ActivationFunctionType.Copy,
                        )
                        half ^= 1
                        if half == 0:
                            flush_pair(full=True)
                if half == 1:
                    flush_pair(full=False)

                # ---- tails ------------------------------------------------
                staged = None
                if n_v:
                    w = VS
                    while w > 1:
                        h = w // 2
                        nc.vector.tensor_max(
                            acc_v[:, 0:h, :], acc_v[:, 0:h, :], acc_v[:, h:w, :]
                        )
                        w = h
                    staged = acc_v[:, 0, :]
                direct = acc_d[:, 0, :] if n_d else None

                outt = outs.tile([F_SH, BCH], OUT_DT, tag="outt")
                if direct is not None and staged is not None:
                    nc.vector.tensor_max(outt[:], direct, staged)  # mixed dtype OK
                    src = outt[:]
                elif direct is not None:
                    src = direct
                else:
                    src = staged
                if affine:
                    nc.vector.tensor_scalar(
                        out=outt[:],
                        in0=src,
                        scalar1=sc[:],
                        scalar2=bi[:],
                        op0=mybir.AluOpType.mult,
                        op1=mybir.AluOpType.add,
                    )
                    src = outt[:]
                elif src is not outt[:] and src.dtype != OUT_DT:
                    nc.vector.tensor_copy(out=outt[:], in_=src)
                    src = outt[:]
                nc.sync.dma_start(out=y_d[:, j * BCH : (j + 1) * BCH], in_=src)

    if fixup:
        split_multiwaits(nc)
    return nc


_CACHED_NC = None


def _get_nc():
    global _CACHED_NC
    if _CACHED_NC is None:
        _CACHED_NC = build_nc()
    return _CACHED_NC


def make_in_maps(x, ww, scale, bias):
    x = np.asarray(x)
    ww = np.asarray(ww)
    scale = np.asarray(scale)
    bias = np.asarray(bias)

    xf = np.ascontiguousarray(x.reshape(B, IDIM).T).astype(np.float32)  # (64, 2048)
    wwf = ww.reshape(FDIM, P, IDIM)
    sc = scale.reshape(FDIM).astype(np.float32)
    bi = bias.reshape(FDIM).astype(np.float32)

    in_maps = []
    for k in range(N_CORES):
        wk = wwf[k * F_SH : (k + 1) * F_SH]  # (128, 64, 64) = (f, p, i)
        wt = np.ascontiguousarray(wk.transpose(2, 1, 0)).astype(np.float32)  # (i,p,f)
        in_maps.append(
            {
                "xt": xf,
                "wt": wt,
                "scale": np.ascontiguousarray(
                    sc[k * F_SH : (k + 1) * F_SH].reshape(F_SH, 1)
                ),
                "bias": np.ascontiguousarray(
                    bi[k * F_SH : (k + 1) * F_SH].reshape(F_SH, 1)
                ),
            }
        )
    return in_maps


def kernel(x, ww, scale, bias):
    in_maps = make_in_maps(x, ww, scale, bias)
    trivial_affine = bool(
        np.all(np.asarray(scale) == 1.0) and np.all(np.asarray(bias) == 0.0)
    )
    nc = build_nc(affine=not trivial_affine)
    res = run_bass_kernel_spmd(nc, in_maps, list(range(N_CORES)))
    out = np.empty((FDIM, B), dtype=np.float32)
    for k in range(N_CORES):
        out[k * F_SH : (k + 1) * F_SH] = res.results[k]["y"].astype(np.float32)
    return np.ascontiguousarray(out.T)
